# revision 2
# baseline (speedup 1.0000x reference)
"""GraphConv GNN (4-layer + mean-pool + head) on 8 Trainium2 NeuronCores.

v2 strategy (GPSIMD ap_gather instead of SWDGE dma_gather):
  - Nodes relabeled into 8 shards x 50 windows x 128 slots; edges bucketed
    per (dst-window, src-core) cell, padded to a per-window quota Qw.
  - Per layer, the aggregation table (fp8, scaled) is stored feature-major,
    u32-packed: table[16*o+s, n] = fp8 feats 4s..4s+4 of src-core-o node n.
  - ap_gather (Q7 ucode, ~1 cycle/idx) pulls edge columns into SBUF; one
    f32 PE transpose per 128-col block flips them edge-major (bit-exact);
    fp8 DoubleRow matmuls with host-built one-hot dmats scatter-add into
    PSUM; root term + bias/ReLU fold per window.
  - AllGather shares each layer's table; head/mean-pool finish on host.
"""

import sys

if "/opt/trn_rl_repo" not in sys.path:
    sys.path.insert(0, "/opt/trn_rl_repo")

import numpy as np
import ml_dtypes

FP8 = ml_dtypes.float8_e4m3
BF16 = ml_dtypes.bfloat16


def _ensure_ntff_hook_module():
    try:
        import antenv.axon_hooks  # noqa: F401
        return
    except ImportError:
        pass
    import types

    mod = types.ModuleType("antenv.axon_hooks")
    mod._hook = None

    def set_axon_ntff_profile_hook(hook):
        mod._hook = hook

    def get_axon_ntff_profile_hook():
        if mod._hook is None:
            try:
                from trn_agent_boot.trn_boot import _ntff_profile_via_ctypes

                mod._hook = _ntff_profile_via_ctypes("/opt/axon/libaxon_pjrt.so")
            except Exception:
                return None
        return mod._hook

    mod.set_axon_ntff_profile_hook = set_axon_ntff_profile_hook
    mod.get_axon_ntff_profile_hook = get_axon_ntff_profile_hook
    try:
        import antenv

        antenv.axon_hooks = mod
    except ImportError:
        pass
    sys.modules["antenv.axon_hooks"] = mod


_ensure_ntff_hook_module()

CORES = 8
WIN = 128
HID = 64
_VER = 12  # bump on every kernel edit: busts the stale PJRT executable cache

_PROGRAM_CACHE: dict = {}
_ONE8 = int(np.float32(1.0).astype(FP8).tobytes()[0])


# --------------------------------------------------------------------------
# Host-side planning
# --------------------------------------------------------------------------
def _forward_scales(x, src, dst, inp):
    """Reference forward on host (sparse) to pick per-table fp8 scales."""
    import scipy.sparse as sp

    n = x.shape[0]
    A = sp.csr_matrix((np.ones(len(src), np.float32), (dst, src)),
                      shape=(n, n))
    w = {k: np.asarray(v, np.float32) for k, v in inp.items()
         if k.startswith(("w", "b", "head"))}
    p1 = x @ w["w1_rel"]
    h1 = np.maximum(A @ p1 + x @ w["w1_root"] + w["b1"], 0.0)
    h2 = np.maximum((A @ h1) @ w["w2_rel"] + h1 @ w["w2_root"] + w["b2"], 0.0)
    h3 = np.maximum((A @ h2) @ w["w3_rel"] + h2 @ w["w3_root"] + w["b3"], 0.0)
    p4 = h3 @ w["w4_rel"]

    def sc(t):
        m = float(np.abs(t).max())
        if m <= 0:
            return 1.0
        return float(2.0 ** np.floor(np.log2(120.0 / m)))

    return sc(p1), sc(h1), sc(h2), sc(p4)


def _plan(x, src, dst, batch, n_graphs):
    n = x.shape[0]
    wpc = -(-n // (CORES * WIN)) + 1  # +1 window of slack for balancing
    sh = wpc * WIN  # slots per core

    indeg = np.bincount(dst, minlength=n).astype(np.int64)
    outdeg = np.bincount(src, minlength=n).astype(np.int64)

    # ---- 1. nodes -> cores (balance in+out load, capacity sh) ------------
    order = np.argsort(-(indeg + outdeg), kind="stable")
    core_of = np.empty(n, np.int64)
    cin = np.zeros(CORES)
    cout = np.zeros(CORES)
    ccnt = np.zeros(CORES, np.int64)
    for v in order:
        score = np.maximum(cin + indeg[v], cout + outdeg[v]) \
            + 0.3 * (cin + indeg[v]) + 0.3 * (cout + outdeg[v])
        score[ccnt >= sh] = np.inf
        c = int(np.argmin(score))
        core_of[v] = c
        cin[c] += indeg[v]
        cout[c] += outdeg[v]
        ccnt[c] += 1

    # per-node in-degree split by src core
    ecore_src = core_of[src]
    deg_o = np.zeros((n, CORES), np.int64)
    np.add.at(deg_o, (dst, ecore_src), 1)

    # ---- 2. windows within core (balance per-(w, o) cells) ---------------
    win_of = np.empty(n, np.int64)
    slot_of = np.empty(n, np.int64)
    for c in range(CORES):
        nodes = np.where(core_of == c)[0]
        nodes = nodes[np.argsort(-indeg[nodes], kind="stable")]
        cell = np.zeros((wpc, CORES), np.int64)
        wcnt = np.zeros(wpc, np.int64)
        for v in nodes:
            cand = cell + deg_o[v][None, :]
            pen = cand.max(axis=1) + 1e-3 * cand.sum(axis=1)
            pen[wcnt >= WIN] = np.inf
            wsel = int(np.argmin(pen))
            win_of[v] = wsel
            slot_of[v] = wcnt[wsel]
            cell[wsel] += deg_o[v]
            wcnt[wsel] += 1

    lid = win_of * WIN + slot_of  # local id within core shard

    # ---- 3. per-window quotas (max cell across cores) --------------------
    ecore_dst = core_of[dst]
    ewin = win_of[dst]
    cellcnt = np.zeros((CORES, wpc, CORES), np.int64)  # dstcore, w, srccore
    np.add.at(cellcnt, (ecore_dst, ewin, ecore_src), 1)
    maxcell = cellcnt.max(axis=(0, 2))  # per window
    KT = np.maximum(-(-maxcell // WIN), 1)
    KT = KT + (KT % 2)  # round up to even: pure-DoubleRow scatter
    Qw = KT * WIN
    woff = np.zeros(wpc + 1, np.int64)
    np.cumsum(Qw, out=woff[1:])
    NI = int(woff[-1])
    assert NI % 16 == 0

    # dmat column layout: per (w, o): KT[w] chunks of 128 cols
    dm_off = np.zeros((wpc, CORES), np.int64)
    o_ = 0
    for w in range(wpc):
        for o in range(CORES):
            dm_off[w, o] = o_
            o_ += KT[w] * WIN
    DMCOLS = o_

    # ---- 4. edge slot assignment ----------------------------------------
    # rank within cell
    cell_key = (ecore_dst * wpc + ewin) * CORES + ecore_src
    eorder = np.argsort(cell_key, kind="stable")
    sk = cell_key[eorder]
    starts = np.zeros(CORES * wpc * CORES + 1, np.int64)
    np.cumsum(np.bincount(cell_key, minlength=CORES * wpc * CORES),
              out=starts[1:])
    rank = np.arange(len(eorder)) - starts[sk]

    # gather list position: per dst-core, per src-core o list:
    pos = woff[ewin[eorder]] + rank  # position within o's list
    idx16 = np.zeros((CORES, CORES, NI), np.int16)  # dstcore, o, pos
    idx16[ecore_dst[eorder], ecore_src[eorder], pos] = \
        lid[src][eorder].astype(np.int16)
    # dmat bytes
    dmat = np.zeros((CORES, 128, DMCOLS), np.uint8)
    rows = (rank % WIN).astype(np.int64)
    cols = dm_off[ewin[eorder], ecore_src[eorder]] \
        + (rank // WIN) * WIN + slot_of[dst][eorder]
    dmat[ecore_dst[eorder], rows, cols] = _ONE8

    # wrap idx lists: [128, NI//16]: idx i of group o -> [16o + i%16, i//16]
    idx_w = np.zeros((CORES, 128, NI // 16), np.int16)
    for c in range(CORES):
        for o in range(CORES):
            idx_w[c, 16 * o:16 * o + 16, :] = \
                idx16[c, o].reshape(NI // 16, 16).T

    # ---- 5. gather groups (windows packed to <= cap cols) ---------------
    def mkgroups(cap):
        gs = []
        cur = [0]
        acc = int(Qw[0])
        for w in range(1, wpc):
            if acc + Qw[w] > cap:
                gs.append(cur)
                cur = [w]
                acc = int(Qw[w])
            else:
                cur.append(w)
                acc += int(Qw[w])
        gs.append(cur)
        return gs

    groups = mkgroups(2560)
    groups3 = mkgroups(1280)

    # ---- 6. per-core inputs ---------------------------------------------
    xT = np.zeros((CORES, x.shape[1], sh), np.float32)
    xT[core_of, :, lid] = x
    bpool = np.zeros((CORES, sh, n_graphs), np.float32)
    bpool[core_of, lid, batch] = 1.0

    return dict(
        wpc=wpc, sh=sh, NI=NI, KT=KT.tolist(), Qw=Qw.tolist(),
        woff=woff.tolist(), DMCOLS=DMCOLS, groups=groups, groups3=groups3,
        dm_off=dm_off, idx_w=idx_w, dmat=dmat, xT=xT, bpool=bpool,
        core_of=core_of, lid=lid, n_graphs=n_graphs,
    )


# --------------------------------------------------------------------------
# Bass program
# --------------------------------------------------------------------------
def _build(wpc, KT, groups, groups3, n_graphs, f_in=128):
    import concourse.bacc as bacc
    import concourse.mybir as mybir
    from concourse import tile

    dt = mybir.dt
    f32 = dt.float32
    bf = dt.bfloat16
    f8 = dt.float8e4
    u32 = dt.uint32
    act = mybir.ActivationFunctionType
    alu = mybir.AluOpType
    DR = mybir.MatmulPerfMode.DoubleRow

    sh = wpc * WIN
    Qw = [k * WIN for k in KT]
    woff = np.zeros(wpc + 1, np.int64)
    np.cumsum(Qw, out=woff[1:])
    NI = int(woff[-1])
    dm_off = np.zeros((wpc, CORES), np.int64)
    o_ = 0
    for w in range(wpc):
        for o in range(CORES):
            dm_off[w, o] = o_
            o_ += KT[w] * WIN
    DMCOLS = o_
    gcap = max(max(sum(Qw[w] for w in g) for g in groups),
               2 * max(sum(Qw[w] for w in g) for g in groups3))

    nc = bacc.Bacc("TRN2", target_bir_lowering=False, debug=False,
                   enable_asserts=False, num_devices=CORES)

    def din(name, shape, dtyp=f32):
        return nc.dram_tensor(name, shape, dtyp, kind="ExternalInput").ap()

    idx = din("idx", [128, NI // 16], dt.int16)
    dmat = din("dmat", [128, DMCOLS], dt.uint8)
    identf = din("identf", [128, 128], f32)
    identb = din("identb", [128, 128], bf)
    identu = din("identu", [128, 128], f8)
    xTd = din("xT", [f_in, sh], bf)
    bp = din("bpool", [sh, n_graphs], bf)
    wts = {}
    for nm, shp in [
        ("w1rel", [128, 64]), ("w1root", [128, 64]),
        ("b1", [64, 1]), ("b1f8", [64, 1]),
        ("w2rel", [64, 128]), ("w2root", [64, 128]),
        ("b2", [128, 1]), ("b2f8", [128, 1]),
        ("w3rela", [128, 128]), ("w3relb", [128, 64]),
        ("w3roota", [128, 128]), ("w3rootb", [128, 64]),
        ("b3a", [128, 1]), ("b3b", [64, 1]),
        ("w4rela", [128, 64]), ("w4relb", [64, 64]),
        ("w4roota", [128, 64]), ("w4rootb", [64, 64]),
        ("b4", [64, 1]),
    ]:
        wts[nm] = din(nm, shp, f32 if nm.startswith("b") else bf)
    scal = din("scal", [1, 8])  # act scales per layer (immediates live in
    # python; this is unused placeholder to keep cache key stable)

    pooled = nc.dram_tensor("pooled", [n_graphs, HID], f32,
                            kind="ExternalOutput").ap()
    ver = nc.dram_tensor("ver", [1, _VER], f32,
                         kind="ExternalOutput").ap()


    rg = [list(range(CORES))]
    S = nc._gnn_scales  # (s1, s2, s3, s4)

    with tile.TileContext(nc) as tc:
        with (
            tc.tile_pool(name="dram", bufs=1, space="DRAM") as dram,
            tc.tile_pool(name="const", bufs=1) as cp,
            tc.tile_pool(name="tblp", bufs=1) as tbp,
            tc.tile_pool(name="gat", bufs=2) as gp,
            tc.tile_pool(name="tt", bufs=3) as tp,
            tc.tile_pool(name="dm", bufs=3) as dp,
            tc.tile_pool(name="st", bufs=4) as sp,
            tc.tile_pool(name="ps_t", bufs=2, space="PSUM") as pst,
            tc.tile_pool(name="ps_a", bufs=2, space="PSUM") as psa,
            tc.tile_pool(name="ps_h", bufs=2, space="PSUM") as psh,
            tc.tile_pool(name="ps_g", bufs=1, space="PSUM") as psg,
        ):
            tin = [dram.tile([16, 2 * sh], bf, name=f"tin{i}")
                   for i in range(5)]  # t1, t2, t3a, t3b, t4
            tsh = [dram.tile([128, 2 * sh], bf, name=f"tsh{i}",
                             addr_space="Shared") for i in range(5)]
            s_idx = cp.tile([128, NI // 16], dt.int16)
            nc.sync.dma_start(s_idx[:], idx[:])
            s_idf = cp.tile([128, 128], f32)
            nc.sync.dma_start(s_idf[:], identf[:])
            s_idb = cp.tile([128, 128], bf)
            nc.sync.dma_start(s_idb[:], identb[:])
            s_idu = cp.tile([128, 128], f8)
            nc.sync.dma_start(s_idu[:], identu[:])
            W = {}
            for nm, ap_ in wts.items():
                W[nm] = cp.tile(list(ap_.shape),
                                f32 if nm.startswith("b") else bf,
                                name=f"w_{nm}")
                nc.sync.dma_start(W[nm][:], ap_[:])
            # x in bf16 feature-major
            s_xbf = cp.tile([f_in, sh], bf)
            nc.sync.dma_start(s_xbf[:], xTd[:])
            # h tiles (bf16) + fp8 staging
            h1bf = cp.tile([64, sh], bf)
            h2bf = cp.tile([128, sh], bf)
            h3abf = cp.tile([128, sh], bf)
            h3bbf = cp.tile([64, sh], bf)
            h4bf = cp.tile([64, sh], bf)
            p1bf = sp.tile([64, sh], bf, tag="f8", name="p1bf", bufs=1)
            p4bf = None

            def pack_win(hbf, part0, ti, w, s, nm_tag):
                """tin[ti][s, ws] u32 = fp8(hbf*s) feats 4s..4s+4 of window
                w nodes: bf16-transpose -> quantize -> f32-transpose."""
                ws = slice(w * WIN, (w + 1) * WIN)
                psn = pst.tile([128, 64], bf, tag="pst",
                               name=f"pkn{ti}_{w}")
                nc.tensor.transpose(psn[:], hbf[part0:part0 + 64, ws],
                                    s_idb[part0:part0 + 64,
                                          part0:part0 + 64])
                nb = sp.tile([128, 64], bf, tag=nm_tag, name=f"nb{ti}_{w}")
                nc.vector.tensor_copy(nb[:], psn[:])
                n8 = sp.tile([128, 16], u32, tag=nm_tag + "8",
                             name=f"n8{ti}_{w}")
                nc.scalar.activation(n8.bitcast(f8), nb[:], act.Copy,
                                     scale=float(s))
                stg = sp.tile([16, 128], u32, tag=nm_tag + "s",
                              name=f"stg{ti}_{w}")
                stg16 = stg.bitcast(dt.uint16).rearrange(
                    "p (c two) -> p c two", two=2)
                n16 = n8.bitcast(bf).rearrange(
                    "p (c two) -> p c two", two=2)
                for jp in range(2):
                    pss = pst.tile([16, 128], bf, tag="pst",
                                   name=f"pks{ti}_{w}_{jp}")
                    nc.tensor.transpose(pss[:], n16[:, :, jp], s_idb[:])
                    nc.vector.tensor_copy(stg16[:, :, jp],
                                          pss.bitcast(dt.uint16))
                nc.sync.dma_start(tin[ti].bitcast(u32)[:, ws], stg[:])


            def allgather(ti):
                nc.gpsimd.collective_compute(
                    "AllGather", alu.bypass, replica_groups=rg,
                    ins=[tin[ti].opt()], outs=[tsh[ti].opt()])

            # ---- L1 table prepass: p1 = x @ (w1_rel*s1) ------------------
            for w in range(wpc):
                ws = slice(w * WIN, (w + 1) * WIN)
                ps = psh.tile([64, 128], f32, tag="psh", name=f"pp1_{w}")
                nc.tensor.matmul(ps[:], W["w1rel"][:], s_xbf[:, ws],
                                 start=True, stop=True)
                nc.vector.tensor_copy(p1bf[:, ws], ps[:])
                pack_win(p1bf, 0, 0, w, 1.0, "nm0")
            allgather(0)

            s_tbl = None

            def layer(li, ti_list, aggw, tail, tail_accum=False,
                      grp=None):
                """li: layer idx; ti_list: table ids; aggw: 64|128;
                tail(w, ps_agg)"""
                nonlocal s_tbl
                if grp is None:
                    grp = groups
                ntab = len(ti_list)
                s_tbl = tbp.tile([128, sh * ntab], u32, tag="tbl",
                                 name=f"tbl{li}")
                for t_i, ti in enumerate(ti_list):
                    nc.sync.dma_start(
                        s_tbl[:, t_i * sh:(t_i + 1) * sh],
                        tsh[ti].bitcast(u32)[:])
                us = aggw // 4  # u32 stripes per edge (16 or 32)
                pend = []

                def stage_mm(w, Tw, dm_t):
                    kt = KT[w]
                    assert kt % 2 == 0
                    ps = psa.tile([aggw, 128], f32, tag="agg",
                                  name=f"agg{li}_{w}")
                    nmm = kt // 2
                    mi = 0
                    for o in range(CORES):
                        for k2 in range(0, kt, 2):
                            nc.tensor.matmul(
                                ps[:], Tw[:, o, k2:k2 + 2, :].bitcast(f8),
                                dm_t[:, o, k2:k2 + 2, :].bitcast(f8),
                                start=(mi == 0),
                                stop=(mi == CORES * nmm - 1
                                      and not tail_accum),
                                perf_mode=DR, skip_group_check=True)
                            mi += 1
                    tail(w, ps)

                for g in grp:
                    cols = sum(Qw[w] for w in g)
                    G = gp.tile([128, gcap], u32, tag="G",
                                name=f"G{li}_{g[0]}")
                    for t_i in range(ntab):
                        nc.gpsimd.ap_gather(
                            G[:, t_i * cols:(t_i + 1) * cols],
                            s_tbl[:, t_i * sh:(t_i + 1) * sh],
                            s_idx[:, woff[g[0]] // 16:
                                  (woff[g[0]] + cols) // 16],
                            channels=128, num_elems=sh, d=1, num_idxs=cols)
                    Gf32 = G.bitcast(f32)
                    for w in g:
                        kt = KT[w]
                        Tw = tp.tile([128, 8, kt, us], u32, tag="Tw",
                                     name=f"Tw{li}_{w}")
                        for k in range(kt):
                            boff = (woff[w] - woff[g[0]]) // 128 + k
                            for t_i in range(ntab):
                                ps_t = pst.tile([128, 128], f32, tag="pst",
                                                name=f"pt{li}_{w}_{k}_{t_i}")
                                c0 = t_i * cols + boff * 128
                                nc.tensor.transpose(
                                    ps_t[:], Gf32[:, c0:c0 + 128],
                                    s_idf[:])
                                nc.vector.tensor_copy(
                                    Tw.bitcast(u32)[:, :, k,
                                                    t_i * 16:t_i * 16 + 16],
                                    ps_t.bitcast(u32).rearrange(
                                        "p (o s) -> p o s", o=8))
                        dm_t = dp.tile([128, 8, kt, 128], dt.uint8, tag="dm",
                                       name=f"dm{li}_{w}")
                        nc.sync.dma_start(
                            dm_t.rearrange("p a b c -> p (a b c)"),
                            dmat[:, dm_off[w, 0]:dm_off[w, 0]
                                 + 8 * kt * 128])
                        # software pipeline: mms of previous window
                        pend.append((w, Tw, dm_t))
                        if len(pend) > 1:
                            stage_mm(*pend.pop(0))
                for item in pend:
                    stage_mm(*item)

            # ---------------- L1 ----------------
            s1, s2, s3, s4 = S

            def tail1(w, ps):
                ws = slice(w * WIN, (w + 1) * WIN)
                nc.tensor.matmul(ps[:], W["w1root"][:], s_xbf[:, ws],
                                 start=False, stop=True,
                                 skip_group_check=True)
                nc.scalar.activation(h1bf[:, ws], ps[:], act.Relu,
                                     bias=W["b1"][:], scale=1.0 / s1)
                pack_win(h1bf, 0, 1, w, s2, "nm0")

            layer(1, [0], 64, tail1, tail_accum=True)
            allgather(1)

            def tail2(w, ps):
                ws = slice(w * WIN, (w + 1) * WIN)
                ag = sp.tile([64, 128], bf, tag="ag", name=f"ag2_{w}")
                nc.vector.tensor_copy(ag[:], ps[:])
                ph = psh.tile([128, 128], f32, tag="psh", name=f"h2_{w}")
                nc.tensor.matmul(ph[:], W["w2rel"][:], ag[:],
                                 start=True, stop=False)
                nc.tensor.matmul(ph[:], W["w2root"][:], h1bf[:, ws],
                                 start=False, stop=True)
                nc.scalar.activation(h2bf[:, ws], ph[:], act.Relu,
                                     bias=W["b2"][:])
                pack_win(h2bf, 0, 2, w, s3, "nm0")
                pack_win(h2bf, 64, 3, w, s3, "nm1")

            layer(2, [1], 64, tail2)
            allgather(2)
            allgather(3)

            def tail3(w, ps):
                ws = slice(w * WIN, (w + 1) * WIN)
                ag = sp.tile([128, 128], bf, tag="ag3", name=f"ag3_{w}")
                nc.vector.tensor_copy(ag[:], ps[:])
                pa = psh.tile([128, 128], f32, tag="psh", name=f"h3a_{w}")
                nc.tensor.matmul(pa[:], W["w3rela"][:], ag[:],
                                 start=True, stop=False)
                nc.tensor.matmul(pa[:], W["w3roota"][:], h2bf[:, ws],
                                 start=False, stop=True)
                nc.scalar.activation(h3abf[:, ws], pa[:], act.Relu,
                                     bias=W["b3a"][:])
                pb = psh.tile([64, 128], f32, tag="psh", name=f"h3b_{w}")
                nc.tensor.matmul(pb[:], W["w3relb"][:], ag[:],
                                 start=True, stop=False)
                nc.tensor.matmul(pb[:], W["w3rootb"][:], h2bf[:, ws],
                                 start=False, stop=True)
                nc.scalar.activation(h3bbf[:, ws], pb[:], act.Relu,
                                     bias=W["b3b"][:])
                pp = psh.tile([64, 128], f32, tag="psh", name=f"p4_{w}")
                nc.tensor.matmul(pp[:], W["w4rela"][:], h3abf[:, ws],
                                 start=True, stop=False)
                nc.tensor.matmul(pp[:], W["w4relb"][:], h3bbf[:, ws],
                                 start=False, stop=True)
                nc.vector.tensor_copy(p4bf[:, ws], pp[:])
                pack_win(p4bf, 0, 4, w, 1.0, "nm0")

            p4bf = sp.tile([64, sh], bf, tag="f82", name="p4bf", bufs=1)
            layer(3, [2, 3], 128, tail3, grp=groups3)
            allgather(4)

            ps_g = psg.tile([n_graphs, HID], f32)

            def tail4(w, ps):
                ws = slice(w * WIN, (w + 1) * WIN)
                nc.tensor.matmul(ps[:], W["w4roota"][:], h3abf[:, ws],
                                 start=False, stop=False,
                                 skip_group_check=True)
                nc.tensor.matmul(ps[:], W["w4rootb"][:], h3bbf[:, ws],
                                 start=False, stop=True,
                                 skip_group_check=True)
                nc.scalar.activation(h4bf[:, ws], ps[:], act.Identity,
                                     bias=W["b4"][:], scale=1.0 / s4)
                # pooling partial
                pt = psa.tile([128, 64], bf, tag="agg", name=f"h4n_{w}")
                nc.tensor.transpose(pt[:], h4bf[:, ws],
                                    s_idb[0:64, 0:64])
                h4n = sp.tile([128, 64], bf, tag="h4n", name=f"h4s_{w}")
                nc.vector.tensor_copy(h4n[:], pt[:])
                bt = sp.tile([128, n_graphs], bf, tag="bt", name=f"bt_{w}")
                nc.sync.dma_start(bt[:], bp[w * WIN:(w + 1) * WIN, :])
                nc.tensor.matmul(ps_g[:], bt[:], h4n[:],
                                 start=(w == 0), stop=(w == wpc - 1),
                                 skip_group_check=True)

            layer(4, [4], 64, tail4, tail_accum=True)

            s_out = sp.tile([n_graphs, HID], f32, tag="out")
            nc.vector.tensor_copy(s_out[:], ps_g[:])
            nc.sync.dma_start(pooled[:], s_out[:])
            vt = sp.tile([1, _VER], f32, tag="ver", bufs=1)
            nc.vector.tensor_copy(vt[:], s_idf[0:1, 0:_VER])
            nc.sync.dma_start(ver[:], vt[:])

    nc.compile()
    return nc


def _get_program(wpc, KT, groups, groups3, n_graphs, f_in, scales):
    key = (wpc, tuple(KT), tuple(map(tuple, groups)),
           tuple(map(tuple, groups3)), n_graphs, f_in, scales)
    if key not in _PROGRAM_CACHE:
        import concourse.bacc as bacc

        orig_init = bacc.Bacc.__init__

        def patched(self, *a, **k):
            orig_init(self, *a, **k)
            self._gnn_scales = scales

        bacc.Bacc.__init__ = patched
        try:
            _PROGRAM_CACHE[key] = _build(
                wpc, KT, list(map(list, groups)), list(map(list, groups3)),
                n_graphs, f_in)
        finally:
            bacc.Bacc.__init__ = orig_init
    return _PROGRAM_CACHE[key]


# --------------------------------------------------------------------------
# Execution
# --------------------------------------------------------------------------
def _in_maps(plan, inputs, scales):
    s1, s2, s3, s4 = scales
    f = {k: np.asarray(v, np.float32) for k, v in inputs.items()
         if k.startswith(("w", "b", "head"))}
    identf = np.eye(128, dtype=np.float32)
    com = {
        "identf": identf,
        "identb": identf.astype(BF16),
        "identu": identf.astype(FP8),
        "scal": np.zeros((1, 8), np.float32),
        "w1rel": (f["w1_rel"] * s1).astype(BF16),
        "w1root": (f["w1_root"] * s1).astype(BF16),
        "b1": f["b1"].reshape(-1, 1),
        "b1f8": (f["b1"] * s2).reshape(-1, 1).astype(np.float32),
        "w2rel": (f["w2_rel"] / s2).astype(BF16),
        "w2root": f["w2_root"].astype(BF16),
        "b2": f["b2"].reshape(-1, 1),
        "b2f8": (f["b2"] * s3).reshape(-1, 1).astype(np.float32),
        "w3rela": (f["w3_rel"][:, 0:128] / s3).astype(BF16),
        "w3relb": (f["w3_rel"][:, 128:192] / s3).astype(BF16),
        "w3roota": f["w3_root"][:, 0:128].astype(BF16),
        "w3rootb": f["w3_root"][:, 128:192].astype(BF16),
        "b3a": f["b3"][0:128].reshape(-1, 1),
        "b3b": f["b3"][128:192].reshape(-1, 1),
        "w4rela": (f["w4_rel"][0:128] * s4).astype(BF16),
        "w4relb": (f["w4_rel"][128:192] * s4).astype(BF16),
        "w4roota": (f["w4_root"][0:128] * s4).astype(BF16),
        "w4rootb": (f["w4_root"][128:192] * s4).astype(BF16),
        "b4": f["b4"].reshape(-1, 1),
    }
    maps = []
    for c in range(CORES):
        m = dict(com)
        m["idx"] = plan["idx_w"][c]
        m["dmat"] = plan["dmat"][c]
        m["xT"] = plan["xT"][c].astype(BF16)
        m["bpool"] = plan["bpool"][c].astype(BF16)
        maps.append(m)
    return maps


def _post(outs, inputs, n_graphs):
    total = np.zeros((n_graphs, HID), np.float32)
    for o in outs:
        total += np.asarray(o["pooled"], np.float32)
    batch = np.asarray(inputs["batch"]).astype(np.int64)
    counts = np.bincount(batch, minlength=n_graphs).astype(np.float32)
    pooled = total / np.maximum(counts, 1.0)[:, None]
    hw = np.asarray(inputs["head_w"], np.float32)
    hb = np.asarray(inputs["head_b"], np.float32)
    return (pooled @ hw + hb).astype(np.float32)


def run(inputs, trace=False, sim=False, n_graphs=64):
    x = np.asarray(inputs["x"], np.float32)
    ei = np.asarray(inputs["edge_index"]).astype(np.int64)
    batch = np.asarray(inputs["batch"]).astype(np.int64)
    src, dst = ei[0], ei[1]
    scales = _forward_scales(x, src, dst, inputs)
    plan = _plan(x, src, dst, batch, n_graphs)
    nc = _get_program(plan["wpc"], tuple(plan["KT"]),
                      tuple(map(tuple, plan["groups"])),
                      tuple(map(tuple, plan["groups3"])), n_graphs,
                      x.shape[1], scales)
    maps = _in_maps(plan, inputs, scales)

    if sim:
        from concourse.bass_interp import MultiCoreSim

        msim = MultiCoreSim(nc, num_cores=CORES)
        for c in range(CORES):
            for k, v in maps[c].items():
                msim.cores[c].tensor(k)[:] = v
        msim.simulate()
        outs = [{"pooled": np.array(msim.cores[c].tensor("pooled"))}
                for c in range(CORES)]
        return _post(outs, inputs, n_graphs), None

    from concourse import bass_utils

    res = bass_utils.run_bass_kernel_spmd(
        nc, maps, core_ids=list(range(CORES)), trace=trace)
    out = _post(res.results, inputs, n_graphs)
    return out, res


def kernel(**inputs) -> np.ndarray:
    out, _ = run(inputs)
    return out


# revision 3
# speedup vs baseline: 1.0061x; 1.0061x over previous
"""GraphConv GNN (4-layer + mean-pool + head) on 8 Trainium2 NeuronCores.

v2 strategy (GPSIMD ap_gather instead of SWDGE dma_gather):
  - Nodes relabeled into 8 shards x 50 windows x 128 slots; edges bucketed
    per (dst-window, src-core) cell, padded to a per-window quota Qw.
  - Per layer, the aggregation table (fp8, scaled) is stored feature-major,
    u32-packed: table[16*o+s, n] = fp8 feats 4s..4s+4 of src-core-o node n.
  - ap_gather (Q7 ucode, ~1 cycle/idx) pulls edge columns into SBUF; one
    f32 PE transpose per 128-col block flips them edge-major (bit-exact);
    fp8 DoubleRow matmuls with host-built one-hot dmats scatter-add into
    PSUM; root term + bias/ReLU fold per window.
  - AllGather shares each layer's table; head/mean-pool finish on host.
"""

import sys

if "/opt/trn_rl_repo" not in sys.path:
    sys.path.insert(0, "/opt/trn_rl_repo")

import numpy as np
import ml_dtypes

FP8 = ml_dtypes.float8_e4m3
BF16 = ml_dtypes.bfloat16


def _ensure_ntff_hook_module():
    try:
        import antenv.axon_hooks  # noqa: F401
        return
    except ImportError:
        pass
    import types

    mod = types.ModuleType("antenv.axon_hooks")
    mod._hook = None

    def set_axon_ntff_profile_hook(hook):
        mod._hook = hook

    def get_axon_ntff_profile_hook():
        if mod._hook is None:
            try:
                from trn_agent_boot.trn_boot import _ntff_profile_via_ctypes

                mod._hook = _ntff_profile_via_ctypes("/opt/axon/libaxon_pjrt.so")
            except Exception:
                return None
        return mod._hook

    mod.set_axon_ntff_profile_hook = set_axon_ntff_profile_hook
    mod.get_axon_ntff_profile_hook = get_axon_ntff_profile_hook
    try:
        import antenv

        antenv.axon_hooks = mod
    except ImportError:
        pass
    sys.modules["antenv.axon_hooks"] = mod


_ensure_ntff_hook_module()

CORES = 8
WIN = 128
HID = 64
_VER = 13  # bump on every kernel edit: busts the stale PJRT executable cache

_PROGRAM_CACHE: dict = {}
_ONE8 = int(np.float32(1.0).astype(FP8).tobytes()[0])


# --------------------------------------------------------------------------
# Host-side planning
# --------------------------------------------------------------------------
def _forward_scales(x, src, dst, inp):
    """Reference forward on host (sparse) to pick per-table fp8 scales."""
    import scipy.sparse as sp

    n = x.shape[0]
    A = sp.csr_matrix((np.ones(len(src), np.float32), (dst, src)),
                      shape=(n, n))
    w = {k: np.asarray(v, np.float32) for k, v in inp.items()
         if k.startswith(("w", "b", "head"))}
    p1 = x @ w["w1_rel"]
    h1 = np.maximum(A @ p1 + x @ w["w1_root"] + w["b1"], 0.0)
    h2 = np.maximum((A @ h1) @ w["w2_rel"] + h1 @ w["w2_root"] + w["b2"], 0.0)
    h3 = np.maximum((A @ h2) @ w["w3_rel"] + h2 @ w["w3_root"] + w["b3"], 0.0)
    p4 = h3 @ w["w4_rel"]

    def sc(t):
        m = float(np.abs(t).max())
        if m <= 0:
            return 1.0
        return float(2.0 ** np.floor(np.log2(120.0 / m)))

    return sc(p1), sc(h1), sc(h2), sc(p4)


def _plan(x, src, dst, batch, n_graphs):
    n = x.shape[0]
    wpc = -(-n // (CORES * WIN)) + 1  # +1 window of slack for balancing
    sh = wpc * WIN  # slots per core

    indeg = np.bincount(dst, minlength=n).astype(np.int64)
    outdeg = np.bincount(src, minlength=n).astype(np.int64)

    # ---- 1. nodes -> cores (balance in+out load, capacity sh) ------------
    order = np.argsort(-(indeg + outdeg), kind="stable")
    core_of = np.empty(n, np.int64)
    cin = np.zeros(CORES)
    cout = np.zeros(CORES)
    ccnt = np.zeros(CORES, np.int64)
    for v in order:
        score = np.maximum(cin + indeg[v], cout + outdeg[v]) \
            + 0.3 * (cin + indeg[v]) + 0.3 * (cout + outdeg[v])
        score[ccnt >= sh] = np.inf
        c = int(np.argmin(score))
        core_of[v] = c
        cin[c] += indeg[v]
        cout[c] += outdeg[v]
        ccnt[c] += 1

    # per-node in-degree split by src core
    ecore_src = core_of[src]
    deg_o = np.zeros((n, CORES), np.int64)
    np.add.at(deg_o, (dst, ecore_src), 1)

    # ---- 2. windows within core (balance per-(w, o) cells) ---------------
    win_of = np.empty(n, np.int64)
    slot_of = np.empty(n, np.int64)
    for c in range(CORES):
        nodes = np.where(core_of == c)[0]
        nodes = nodes[np.argsort(-indeg[nodes], kind="stable")]
        cell = np.zeros((wpc, CORES), np.int64)
        wcnt = np.zeros(wpc, np.int64)
        for v in nodes:
            cand = cell + deg_o[v][None, :]
            pen = cand.max(axis=1) + 1e-3 * cand.sum(axis=1)
            pen[wcnt >= WIN] = np.inf
            wsel = int(np.argmin(pen))
            win_of[v] = wsel
            slot_of[v] = wcnt[wsel]
            cell[wsel] += deg_o[v]
            wcnt[wsel] += 1

    lid = win_of * WIN + slot_of  # local id within core shard

    # ---- 3. per-window quotas (max cell across cores) --------------------
    ecore_dst = core_of[dst]
    ewin = win_of[dst]
    cellcnt = np.zeros((CORES, wpc, CORES), np.int64)  # dstcore, w, srccore
    np.add.at(cellcnt, (ecore_dst, ewin, ecore_src), 1)
    maxcell = cellcnt.max(axis=(0, 2))  # per window
    KT = np.maximum(-(-maxcell // WIN), 1)
    KT = KT + (KT % 2)  # round up to even: pure-DoubleRow scatter
    Qw = KT * WIN
    woff = np.zeros(wpc + 1, np.int64)
    np.cumsum(Qw, out=woff[1:])
    NI = int(woff[-1])
    assert NI % 16 == 0

    # dmat column layout: per (w, o): KT[w] chunks of 128 cols
    dm_off = np.zeros((wpc, CORES), np.int64)
    o_ = 0
    for w in range(wpc):
        for o in range(CORES):
            dm_off[w, o] = o_
            o_ += KT[w] * WIN
    DMCOLS = o_

    # ---- 4. edge slot assignment ----------------------------------------
    # rank within cell
    cell_key = (ecore_dst * wpc + ewin) * CORES + ecore_src
    eorder = np.argsort(cell_key, kind="stable")
    sk = cell_key[eorder]
    starts = np.zeros(CORES * wpc * CORES + 1, np.int64)
    np.cumsum(np.bincount(cell_key, minlength=CORES * wpc * CORES),
              out=starts[1:])
    rank = np.arange(len(eorder)) - starts[sk]

    # gather list position: per dst-core, per src-core o list:
    pos = woff[ewin[eorder]] + rank  # position within o's list
    idx16 = np.zeros((CORES, CORES, NI), np.int16)  # dstcore, o, pos
    idx16[ecore_dst[eorder], ecore_src[eorder], pos] = \
        lid[src][eorder].astype(np.int16)
    # dmat bytes
    dmat = np.zeros((CORES, 128, DMCOLS), np.uint8)
    rows = (rank % WIN).astype(np.int64)
    cols = dm_off[ewin[eorder], ecore_src[eorder]] \
        + (rank // WIN) * WIN + slot_of[dst][eorder]
    dmat[ecore_dst[eorder], rows, cols] = _ONE8

    # wrap idx lists: [128, NI//16]: idx i of group o -> [16o + i%16, i//16]
    idx_w = np.zeros((CORES, 128, NI // 16), np.int16)
    for c in range(CORES):
        for o in range(CORES):
            idx_w[c, 16 * o:16 * o + 16, :] = \
                idx16[c, o].reshape(NI // 16, 16).T

    # ---- 5. gather groups (windows packed to <= cap cols) ---------------
    def mkgroups(cap):
        gs = []
        cur = [0]
        acc = int(Qw[0])
        for w in range(1, wpc):
            if acc + Qw[w] > cap:
                gs.append(cur)
                cur = [w]
                acc = int(Qw[w])
            else:
                cur.append(w)
                acc += int(Qw[w])
        gs.append(cur)
        return gs

    groups = mkgroups(2560)
    groups3 = mkgroups(1280)

    # ---- 6. per-core inputs ---------------------------------------------
    xT = np.zeros((CORES, x.shape[1], sh), np.float32)
    xT[core_of, :, lid] = x
    bpool = np.zeros((CORES, sh, n_graphs), np.float32)
    bpool[core_of, lid, batch] = 1.0

    return dict(
        wpc=wpc, sh=sh, NI=NI, KT=KT.tolist(), Qw=Qw.tolist(),
        woff=woff.tolist(), DMCOLS=DMCOLS, groups=groups, groups3=groups3,
        dm_off=dm_off, idx_w=idx_w, dmat=dmat, xT=xT, bpool=bpool,
        core_of=core_of, lid=lid, n_graphs=n_graphs,
    )


# --------------------------------------------------------------------------
# Bass program
# --------------------------------------------------------------------------
def _build(wpc, KT, groups, groups3, n_graphs, f_in=128):
    import concourse.bacc as bacc
    import concourse.mybir as mybir
    from concourse import tile

    dt = mybir.dt
    f32 = dt.float32
    bf = dt.bfloat16
    f8 = dt.float8e4
    u32 = dt.uint32
    act = mybir.ActivationFunctionType
    alu = mybir.AluOpType
    DR = mybir.MatmulPerfMode.DoubleRow

    sh = wpc * WIN
    Qw = [k * WIN for k in KT]
    woff = np.zeros(wpc + 1, np.int64)
    np.cumsum(Qw, out=woff[1:])
    NI = int(woff[-1])
    dm_off = np.zeros((wpc, CORES), np.int64)
    o_ = 0
    for w in range(wpc):
        for o in range(CORES):
            dm_off[w, o] = o_
            o_ += KT[w] * WIN
    DMCOLS = o_
    gcap = max(max(sum(Qw[w] for w in g) for g in groups),
               2 * max(sum(Qw[w] for w in g) for g in groups3))

    nc = bacc.Bacc("TRN2", target_bir_lowering=False, debug=False,
                   enable_asserts=False, num_devices=CORES)

    def din(name, shape, dtyp=f32):
        return nc.dram_tensor(name, shape, dtyp, kind="ExternalInput").ap()

    idx = din("idx", [128, NI // 16], dt.int16)
    dmat = din("dmat", [128, DMCOLS], dt.uint8)
    identf = din("identf", [128, 128], f32)
    identb = din("identb", [128, 128], bf)
    identu = din("identu", [128, 128], f8)
    xTd = din("xT", [f_in, sh], bf)
    bp = din("bpool", [sh, n_graphs], bf)
    wts = {}
    for nm, shp in [
        ("w1rel", [128, 64]), ("w1root", [128, 64]),
        ("b1", [64, 1]), ("b1f8", [64, 1]),
        ("w2rel", [64, 128]), ("w2root", [64, 128]),
        ("b2", [128, 1]), ("b2f8", [128, 1]),
        ("w3rela", [128, 128]), ("w3relb", [128, 64]),
        ("w3roota", [128, 128]), ("w3rootb", [128, 64]),
        ("b3a", [128, 1]), ("b3b", [64, 1]),
        ("w4rela", [128, 64]), ("w4relb", [64, 64]),
        ("w4roota", [128, 64]), ("w4rootb", [64, 64]),
        ("b4", [64, 1]),
    ]:
        wts[nm] = din(nm, shp, f32 if nm.startswith("b") else bf)
    scal = din("scal", [1, 8])  # act scales per layer (immediates live in
    # python; this is unused placeholder to keep cache key stable)

    pooled = nc.dram_tensor("pooled", [n_graphs, HID], f32,
                            kind="ExternalOutput").ap()
    ver = nc.dram_tensor("ver", [1, _VER], f32,
                         kind="ExternalOutput").ap()


    rg = [list(range(CORES))]
    S = nc._gnn_scales  # (s1, s2, s3, s4)

    with tile.TileContext(nc) as tc:
        with (
            tc.tile_pool(name="dram", bufs=1, space="DRAM") as dram,
            tc.tile_pool(name="const", bufs=1) as cp,
            tc.tile_pool(name="tblp", bufs=1) as tbp,
            tc.tile_pool(name="gat", bufs=2) as gp,
            tc.tile_pool(name="tt", bufs=4) as tp,
            tc.tile_pool(name="dm", bufs=4) as dp,
            tc.tile_pool(name="st", bufs=4) as sp,
            tc.tile_pool(name="ps_t", bufs=3, space="PSUM") as pst,
            tc.tile_pool(name="ps_a", bufs=2, space="PSUM") as psa,
            tc.tile_pool(name="ps_h", bufs=2, space="PSUM") as psh,
            tc.tile_pool(name="ps_g", bufs=1, space="PSUM") as psg,
        ):
            tin = [dram.tile([16, 2 * sh], bf, name=f"tin{i}")
                   for i in range(5)]  # t1, t2, t3a, t3b, t4
            tsh = [dram.tile([128, 2 * sh], bf, name=f"tsh{i}",
                             addr_space="Shared") for i in range(5)]
            s_idx = cp.tile([128, NI // 16], dt.int16)
            nc.sync.dma_start(s_idx[:], idx[:])
            s_idf = cp.tile([128, 128], f32)
            nc.sync.dma_start(s_idf[:], identf[:])
            s_idb = cp.tile([128, 128], bf)
            nc.sync.dma_start(s_idb[:], identb[:])
            s_idu = cp.tile([128, 128], f8)
            nc.sync.dma_start(s_idu[:], identu[:])
            W = {}
            for nm, ap_ in wts.items():
                W[nm] = cp.tile(list(ap_.shape),
                                f32 if nm.startswith("b") else bf,
                                name=f"w_{nm}")
                nc.sync.dma_start(W[nm][:], ap_[:])
            # x in bf16 feature-major
            s_xbf = cp.tile([f_in, sh], bf)
            nc.sync.dma_start(s_xbf[:], xTd[:])
            # h tiles (bf16) + fp8 staging
            h1bf = cp.tile([64, sh], bf)
            h2bf = cp.tile([128, sh], bf)
            h3abf = cp.tile([128, sh], bf)
            h3bbf = cp.tile([64, sh], bf)
            h4bf = cp.tile([64, sh], bf)
            p1bf = sp.tile([64, sh], bf, tag="f8", name="p1bf", bufs=1)
            p4bf = None

            def pack_win(hbf, part0, ti, w, s, nm_tag):
                """tin[ti][s, ws] u32 = fp8(hbf*s) feats 4s..4s+4 of window
                w nodes: bf16-transpose -> quantize -> f32-transpose."""
                ws = slice(w * WIN, (w + 1) * WIN)
                psn = pst.tile([128, 64], bf, tag="pst",
                               name=f"pkn{ti}_{w}")
                nc.tensor.transpose(psn[:], hbf[part0:part0 + 64, ws],
                                    s_idb[part0:part0 + 64,
                                          part0:part0 + 64])
                nb = sp.tile([128, 64], bf, tag=nm_tag, name=f"nb{ti}_{w}")
                nc.vector.tensor_copy(nb[:], psn[:])
                n8 = sp.tile([128, 16], u32, tag=nm_tag + "8",
                             name=f"n8{ti}_{w}")
                nc.scalar.activation(n8.bitcast(f8), nb[:], act.Copy,
                                     scale=float(s))
                stg = sp.tile([16, 128], u32, tag=nm_tag + "s",
                              name=f"stg{ti}_{w}")
                stg16 = stg.bitcast(dt.uint16).rearrange(
                    "p (c two) -> p c two", two=2)
                n16 = n8.bitcast(bf).rearrange(
                    "p (c two) -> p c two", two=2)
                for jp in range(2):
                    pss = pst.tile([16, 128], bf, tag="pst",
                                   name=f"pks{ti}_{w}_{jp}")
                    nc.tensor.transpose(pss[:], n16[:, :, jp], s_idb[:])
                    nc.vector.tensor_copy(stg16[:, :, jp],
                                          pss.bitcast(dt.uint16))
                nc.sync.dma_start(tin[ti].bitcast(u32)[:, ws], stg[:])


            def allgather(ti):
                nc.gpsimd.collective_compute(
                    "AllGather", alu.bypass, replica_groups=rg,
                    ins=[tin[ti].opt()], outs=[tsh[ti].opt()])

            # ---- L1 table prepass: p1 = x @ (w1_rel*s1) ------------------
            for w in range(wpc):
                ws = slice(w * WIN, (w + 1) * WIN)
                ps = psh.tile([64, 128], f32, tag="psh", name=f"pp1_{w}")
                nc.tensor.matmul(ps[:], W["w1rel"][:], s_xbf[:, ws],
                                 start=True, stop=True)
                nc.vector.tensor_copy(p1bf[:, ws], ps[:])
                pack_win(p1bf, 0, 0, w, 1.0, "nm0")
            allgather(0)

            s_tbl = None

            def layer(li, ti_list, aggw, tail, tail_accum=False,
                      grp=None):
                """li: layer idx; ti_list: table ids; aggw: 64|128;
                tail(w, ps_agg)"""
                nonlocal s_tbl
                if grp is None:
                    grp = groups
                ntab = len(ti_list)
                s_tbl = tbp.tile([128, sh * ntab], u32, tag="tbl",
                                 name=f"tbl{li}")
                for t_i, ti in enumerate(ti_list):
                    nc.sync.dma_start(
                        s_tbl[:, t_i * sh:(t_i + 1) * sh],
                        tsh[ti].bitcast(u32)[:])
                us = aggw // 4  # u32 stripes per edge (16 or 32)
                pend = []

                def stage_mm(w, Tw, dm_t):
                    kt = KT[w]
                    assert kt % 2 == 0
                    ps = psa.tile([aggw, 128], f32, tag="agg",
                                  name=f"agg{li}_{w}")
                    nmm = kt // 2
                    mi = 0
                    for o in range(CORES):
                        for k2 in range(0, kt, 2):
                            nc.tensor.matmul(
                                ps[:], Tw[:, o, k2:k2 + 2, :].bitcast(f8),
                                dm_t[:, o, k2:k2 + 2, :].bitcast(f8),
                                start=(mi == 0),
                                stop=(mi == CORES * nmm - 1
                                      and not tail_accum),
                                perf_mode=DR, skip_group_check=True)
                            mi += 1
                    tail(w, ps)

                for g in grp:
                    cols = sum(Qw[w] for w in g)
                    G = gp.tile([128, gcap], u32, tag="G",
                                name=f"G{li}_{g[0]}")
                    for t_i in range(ntab):
                        nc.gpsimd.ap_gather(
                            G[:, t_i * cols:(t_i + 1) * cols],
                            s_tbl[:, t_i * sh:(t_i + 1) * sh],
                            s_idx[:, woff[g[0]] // 16:
                                  (woff[g[0]] + cols) // 16],
                            channels=128, num_elems=sh, d=1, num_idxs=cols)
                    Gf32 = G.bitcast(f32)
                    for w in g:
                        kt = KT[w]
                        Tw = tp.tile([128, 8, kt, us], u32, tag="Tw",
                                     name=f"Tw{li}_{w}")
                        for k in range(kt):
                            boff = (woff[w] - woff[g[0]]) // 128 + k
                            for t_i in range(ntab):
                                ps_t = pst.tile([128, 128], f32, tag="pst",
                                                name=f"pt{li}_{w}_{k}_{t_i}")
                                c0 = t_i * cols + boff * 128
                                nc.tensor.transpose(
                                    ps_t[:], Gf32[:, c0:c0 + 128],
                                    s_idf[:])
                                nc.vector.tensor_copy(
                                    Tw.bitcast(u32)[:, :, k,
                                                    t_i * 16:t_i * 16 + 16],
                                    ps_t.bitcast(u32).rearrange(
                                        "p (o s) -> p o s", o=8))
                        dm_t = dp.tile([128, 8, kt, 128], dt.uint8, tag="dm",
                                       name=f"dm{li}_{w}")
                        nc.sync.dma_start(
                            dm_t.rearrange("p a b c -> p (a b c)"),
                            dmat[:, dm_off[w, 0]:dm_off[w, 0]
                                 + 8 * kt * 128])
                        # software pipeline: mms of previous window
                        pend.append((w, Tw, dm_t))
                        if len(pend) > 2:
                            stage_mm(*pend.pop(0))
                for item in pend:
                    stage_mm(*item)

            # ---------------- L1 ----------------
            s1, s2, s3, s4 = S

            def tail1(w, ps):
                ws = slice(w * WIN, (w + 1) * WIN)
                nc.tensor.matmul(ps[:], W["w1root"][:], s_xbf[:, ws],
                                 start=False, stop=True,
                                 skip_group_check=True)
                nc.scalar.activation(h1bf[:, ws], ps[:], act.Relu,
                                     bias=W["b1"][:], scale=1.0 / s1)
                pack_win(h1bf, 0, 1, w, s2, "nm0")

            layer(1, [0], 64, tail1, tail_accum=True)
            allgather(1)

            def tail2(w, ps):
                ws = slice(w * WIN, (w + 1) * WIN)
                ag = sp.tile([64, 128], bf, tag="ag", name=f"ag2_{w}")
                nc.vector.tensor_copy(ag[:], ps[:])
                ph = psh.tile([128, 128], f32, tag="psh", name=f"h2_{w}")
                nc.tensor.matmul(ph[:], W["w2rel"][:], ag[:],
                                 start=True, stop=False)
                nc.tensor.matmul(ph[:], W["w2root"][:], h1bf[:, ws],
                                 start=False, stop=True)
                nc.scalar.activation(h2bf[:, ws], ph[:], act.Relu,
                                     bias=W["b2"][:])
                pack_win(h2bf, 0, 2, w, s3, "nm0")
                pack_win(h2bf, 64, 3, w, s3, "nm1")

            layer(2, [1], 64, tail2)
            allgather(2)
            allgather(3)

            def tail3(w, ps):
                ws = slice(w * WIN, (w + 1) * WIN)
                ag = sp.tile([128, 128], bf, tag="ag3", name=f"ag3_{w}")
                nc.vector.tensor_copy(ag[:], ps[:])
                pa = psh.tile([128, 128], f32, tag="psh", name=f"h3a_{w}")
                nc.tensor.matmul(pa[:], W["w3rela"][:], ag[:],
                                 start=True, stop=False)
                nc.tensor.matmul(pa[:], W["w3roota"][:], h2bf[:, ws],
                                 start=False, stop=True)
                nc.scalar.activation(h3abf[:, ws], pa[:], act.Relu,
                                     bias=W["b3a"][:])
                pb = psh.tile([64, 128], f32, tag="psh", name=f"h3b_{w}")
                nc.tensor.matmul(pb[:], W["w3relb"][:], ag[:],
                                 start=True, stop=False)
                nc.tensor.matmul(pb[:], W["w3rootb"][:], h2bf[:, ws],
                                 start=False, stop=True)
                nc.scalar.activation(h3bbf[:, ws], pb[:], act.Relu,
                                     bias=W["b3b"][:])
                pp = psh.tile([64, 128], f32, tag="psh", name=f"p4_{w}")
                nc.tensor.matmul(pp[:], W["w4rela"][:], h3abf[:, ws],
                                 start=True, stop=False)
                nc.tensor.matmul(pp[:], W["w4relb"][:], h3bbf[:, ws],
                                 start=False, stop=True)
                nc.vector.tensor_copy(p4bf[:, ws], pp[:])
                pack_win(p4bf, 0, 4, w, 1.0, "nm0")

            p4bf = sp.tile([64, sh], bf, tag="f82", name="p4bf", bufs=1)
            layer(3, [2, 3], 128, tail3, grp=groups3)
            allgather(4)

            ps_g = psg.tile([n_graphs, HID], f32)

            def tail4(w, ps):
                ws = slice(w * WIN, (w + 1) * WIN)
                nc.tensor.matmul(ps[:], W["w4roota"][:], h3abf[:, ws],
                                 start=False, stop=False,
                                 skip_group_check=True)
                nc.tensor.matmul(ps[:], W["w4rootb"][:], h3bbf[:, ws],
                                 start=False, stop=True,
                                 skip_group_check=True)
                nc.scalar.activation(h4bf[:, ws], ps[:], act.Identity,
                                     bias=W["b4"][:], scale=1.0 / s4)
                # pooling partial
                pt = psa.tile([128, 64], bf, tag="agg", name=f"h4n_{w}")
                nc.tensor.transpose(pt[:], h4bf[:, ws],
                                    s_idb[0:64, 0:64])
                h4n = sp.tile([128, 64], bf, tag="h4n", name=f"h4s_{w}")
                nc.vector.tensor_copy(h4n[:], pt[:])
                bt = sp.tile([128, n_graphs], bf, tag="bt", name=f"bt_{w}")
                nc.sync.dma_start(bt[:], bp[w * WIN:(w + 1) * WIN, :])
                nc.tensor.matmul(ps_g[:], bt[:], h4n[:],
                                 start=(w == 0), stop=(w == wpc - 1),
                                 skip_group_check=True)

            layer(4, [4], 64, tail4, tail_accum=True)

            s_out = sp.tile([n_graphs, HID], f32, tag="out")
            nc.vector.tensor_copy(s_out[:], ps_g[:])
            nc.sync.dma_start(pooled[:], s_out[:])
            vt = sp.tile([1, _VER], f32, tag="ver", bufs=1)
            nc.vector.tensor_copy(vt[:], s_idf[0:1, 0:_VER])
            nc.sync.dma_start(ver[:], vt[:])

    nc.compile()
    return nc


def _get_program(wpc, KT, groups, groups3, n_graphs, f_in, scales):
    key = (wpc, tuple(KT), tuple(map(tuple, groups)),
           tuple(map(tuple, groups3)), n_graphs, f_in, scales)
    if key not in _PROGRAM_CACHE:
        import concourse.bacc as bacc

        orig_init = bacc.Bacc.__init__

        def patched(self, *a, **k):
            orig_init(self, *a, **k)
            self._gnn_scales = scales

        bacc.Bacc.__init__ = patched
        try:
            _PROGRAM_CACHE[key] = _build(
                wpc, KT, list(map(list, groups)), list(map(list, groups3)),
                n_graphs, f_in)
        finally:
            bacc.Bacc.__init__ = orig_init
    return _PROGRAM_CACHE[key]


# --------------------------------------------------------------------------
# Execution
# --------------------------------------------------------------------------
def _in_maps(plan, inputs, scales):
    s1, s2, s3, s4 = scales
    f = {k: np.asarray(v, np.float32) for k, v in inputs.items()
         if k.startswith(("w", "b", "head"))}
    identf = np.eye(128, dtype=np.float32)
    com = {
        "identf": identf,
        "identb": identf.astype(BF16),
        "identu": identf.astype(FP8),
        "scal": np.zeros((1, 8), np.float32),
        "w1rel": (f["w1_rel"] * s1).astype(BF16),
        "w1root": (f["w1_root"] * s1).astype(BF16),
        "b1": f["b1"].reshape(-1, 1),
        "b1f8": (f["b1"] * s2).reshape(-1, 1).astype(np.float32),
        "w2rel": (f["w2_rel"] / s2).astype(BF16),
        "w2root": f["w2_root"].astype(BF16),
        "b2": f["b2"].reshape(-1, 1),
        "b2f8": (f["b2"] * s3).reshape(-1, 1).astype(np.float32),
        "w3rela": (f["w3_rel"][:, 0:128] / s3).astype(BF16),
        "w3relb": (f["w3_rel"][:, 128:192] / s3).astype(BF16),
        "w3roota": f["w3_root"][:, 0:128].astype(BF16),
        "w3rootb": f["w3_root"][:, 128:192].astype(BF16),
        "b3a": f["b3"][0:128].reshape(-1, 1),
        "b3b": f["b3"][128:192].reshape(-1, 1),
        "w4rela": (f["w4_rel"][0:128] * s4).astype(BF16),
        "w4relb": (f["w4_rel"][128:192] * s4).astype(BF16),
        "w4roota": (f["w4_root"][0:128] * s4).astype(BF16),
        "w4rootb": (f["w4_root"][128:192] * s4).astype(BF16),
        "b4": f["b4"].reshape(-1, 1),
    }
    maps = []
    for c in range(CORES):
        m = dict(com)
        m["idx"] = plan["idx_w"][c]
        m["dmat"] = plan["dmat"][c]
        m["xT"] = plan["xT"][c].astype(BF16)
        m["bpool"] = plan["bpool"][c].astype(BF16)
        maps.append(m)
    return maps


def _post(outs, inputs, n_graphs):
    total = np.zeros((n_graphs, HID), np.float32)
    for o in outs:
        total += np.asarray(o["pooled"], np.float32)
    batch = np.asarray(inputs["batch"]).astype(np.int64)
    counts = np.bincount(batch, minlength=n_graphs).astype(np.float32)
    pooled = total / np.maximum(counts, 1.0)[:, None]
    hw = np.asarray(inputs["head_w"], np.float32)
    hb = np.asarray(inputs["head_b"], np.float32)
    return (pooled @ hw + hb).astype(np.float32)


def run(inputs, trace=False, sim=False, n_graphs=64):
    x = np.asarray(inputs["x"], np.float32)
    ei = np.asarray(inputs["edge_index"]).astype(np.int64)
    batch = np.asarray(inputs["batch"]).astype(np.int64)
    src, dst = ei[0], ei[1]
    scales = _forward_scales(x, src, dst, inputs)
    plan = _plan(x, src, dst, batch, n_graphs)
    nc = _get_program(plan["wpc"], tuple(plan["KT"]),
                      tuple(map(tuple, plan["groups"])),
                      tuple(map(tuple, plan["groups3"])), n_graphs,
                      x.shape[1], scales)
    maps = _in_maps(plan, inputs, scales)

    if sim:
        from concourse.bass_interp import MultiCoreSim

        msim = MultiCoreSim(nc, num_cores=CORES)
        for c in range(CORES):
            for k, v in maps[c].items():
                msim.cores[c].tensor(k)[:] = v
        msim.simulate()
        outs = [{"pooled": np.array(msim.cores[c].tensor("pooled"))}
                for c in range(CORES)]
        return _post(outs, inputs, n_graphs), None

    from concourse import bass_utils

    res = bass_utils.run_bass_kernel_spmd(
        nc, maps, core_ids=list(range(CORES)), trace=trace)
    out = _post(res.results, inputs, n_graphs)
    return out, res


def kernel(**inputs) -> np.ndarray:
    out, _ = run(inputs)
    return out
